# revision 10
# baseline (speedup 1.0000x reference)
"""Trainium2 Bass kernel for nn_MistralSparseMLP (topk_masking).

Self-contained: kernel(**inputs) -> np.ndarray takes the FULL inputs
(x, Wg, Wu, Wd, Wr, Wsv, bsv) and returns the FULL [B, H] output.

v3 strategy (8-way tensor parallel over the intermediate dim I, no
collectives):
  - host prep (pure layout, no compute): x.T, Wr.T, per-core Wd-shard
    transposed to [IS, H]; Wsv/bsv permuted per core so the local shard
    is always probs columns [0, FL) (keeps the SPMD program identical);
    index/identity constants.
  - router: u0 = Wr @ x0 on PE (WrT direct from DRAM), probs =
    relu(Wsv.u0 + bsv) via batched DVE mult+reduce (Wsv replicated).
  - global top-K threshold: branchless 31-step bisection on the fp32
    bit patterns, replicated across lanes; cross-partition counts via
    ones-matmul on PE.
  - compaction: candidates -> gpsimd sparse_gather -> NSLOT slots; pads
    get index 0 for the DMA gather (finite data) and 1e9 for the
    scatter compare (never matches => contribute zero).
  - gate/up: indirect-DMA gather of only the selected rows of Wg/Wu
    (the memory win); PE transpose of the gathered blocks; x-stationary
    matmuls accumulate g/u as [B, slots].
  - mu scatter: selection matrices (is_equal of gather indices vs row
    ids) matmul the compact mu back to dense local rows [c, B].
  - down: WdT shard streams dense [c-major] straight from DRAM (no
    on-device transposes), fp32 matmuls accumulate out[B, 512] in psum
    2-deep over fl, DVE/gpsimd adds into a [B, H] SBUF accumulator.
  - host sums the 8 partial [B, H] outputs.
"""
import sys

sys.path.insert(0, "/opt/trn_rl_repo")

import numpy as np

NCORES = 8

FULL_CFG = dict(H=4096, I=14336, R=128, B=32, K=4096, NT=5, WD_BUFS=4)
BISECT_ITERS = 31


def _derived(cfg):
    H, I, B = cfg["H"], cfg["I"], cfg["B"]
    IS = I // NCORES          # local shard of I
    FI = I // 128             # probs free cols
    FL = IS // 128            # local probs cols
    HK = H // 128             # H chunks
    NT = cfg["NT"]
    NSLOT = NT * 128          # padded compact slots
    assert I % (NCORES * 128) == 0 and H % 128 == 0
    return IS, FI, FL, HK, NT, NSLOT


def _build_nc(cfg):
    from contextlib import ExitStack

    import concourse.bass as bass
    import concourse.bacc as bacc
    import concourse.mybir as mybir
    import concourse.tile as tile

    H, I, R, B, K = cfg["H"], cfg["I"], cfg["R"], cfg["B"], cfg["K"]
    IS, FI, FL, HK, NT, NSLOT = _derived(cfg)
    WD_BUFS = cfg["WD_BUFS"]
    HB = H // 512             # down output column blocks

    dt = mybir.dt
    f32, u32 = dt.float32, dt.uint32
    Alu = mybir.AluOpType
    Act = mybir.ActivationFunctionType
    Axis = mybir.AxisListType

    nc = bacc.Bacc("TRN2", target_bir_lowering=False, debug=False)

    def din(name, shape, d=f32):
        return nc.dram_tensor(name, shape, d, kind="ExternalInput").ap()

    def dout(name, shape, d=f32):
        return nc.dram_tensor(name, shape, d, kind="ExternalOutput").ap()

    t_xT = din("xT", [H, B])
    t_wrT = din("wrT", [H, R])
    t_wsv = din("wsv", [I, R])          # per-core permuted: local shard first
    t_bsv = din("bsv_pf", [128, FI])    # permuted + [p, f] relayout
    t_wg = din("wg", [IS, H])
    t_wu = din("wu", [IS, H])
    t_wdT = din("wdT", [IS, H])         # host-transposed Wd shard
    t_ident = din("ident", [128, 128])
    t_ones = din("ones", [128, 128])
    t_candidx = din("candidx", [128, FL])      # f32 value 128*fl + p
    t_wiota = din("wiota", [16, NSLOT // 16])  # f32 wrapped slot id 16*f + p
    t_coliota = din("col_iota", [128, 128])    # f32 value = column index

    o_out = dout("out", [B, H])
    o_nf = dout("nf", [1, 1], u32)
    o_probs = dout("probs", [128, FI])

    with ExitStack() as ctx:
        tc = ctx.enter_context(tile.TileContext(nc))

        def pool(name, bufs, space="SBUF"):
            return ctx.enter_context(tc.tile_pool(name=name, bufs=bufs, space=space))

        cpool = pool("consts", 1)
        xtpool = pool("xt", 1)
        wrpool = pool("wrc", 2)
        wsvpool = pool("wsv", 3)
        scrpool = pool("scr", 2)
        smallp = pool("small", 1)        # persistent small tiles
        tmpp = pool("tmp", 3)            # transient small tiles
        wtpool = pool("wt", 6)           # transposed gate/up blocks
        wgpool = pool("wgraw", 2)
        wupool = pool("wuraw", 2)
        mutpool = pool("mut", NT)
        mudpool = pool("mud", FL)
        wdtpool = pool("wdt", WD_BUFS)
        oaccpool = pool("oacc", 1)

        # PSUM: 8 banks, statically partitioned by pool/tag.
        ptr = pool("ptr", 2, space="PSUM")      # transposes/scatter (tag "t")
        pga = pool("pga", 2, space="PSUM")      # gate accum + u0 (tag "g")
        pua = pool("pua", 2, space="PSUM")      # up accum + bcasts (tag "u")
        pdo = pool("pdo", 2, space="PSUM")      # down accum + bisect (tag "d")

        # ---------------- constants ----------------
        ident = cpool.tile([128, 128], f32)
        nc.sync.dma_start(ident[:], t_ident)
        ones = cpool.tile([128, 128], f32)
        nc.sync.dma_start(ones[:], t_ones)
        candidx = cpool.tile([128, FL], f32)
        nc.sync.dma_start(candidx[:], t_candidx)
        wiota = cpool.tile([16, NSLOT // 16], f32)
        nc.sync.dma_start(wiota[:], t_wiota)
        bsvc = cpool.tile([128, FI], f32)
        nc.sync.dma_start(bsvc[:], t_bsv)
        col_iota = cpool.tile([128, 128], f32)
        nc.sync.dma_start(col_iota[:], t_coliota)

        # ---------------- xT  [128, HK*B] direct from host x.T -------------
        xT = xtpool.tile([128, HK * B], f32)
        nc.sync.dma_start(xT[:].rearrange("p (h b) -> p h b", h=HK),
                          t_xT.rearrange("(h p) b -> p h b", p=128))

        # ---------------- u0 = Wr @ x0  [R=128, 1] ----------------
        u0ps = pga.tile([128, 32], f32, tag="g")
        for h in range(HK):
            wrc = wrpool.tile([128, R], f32)
            nc.sync.dma_start(wrc[:], t_wrT[128 * h:128 * h + 128, :])
            nc.tensor.matmul(u0ps[:, :1], wrc[:], xT[:, B * h:B * h + 1],
                             start=(h == 0), stop=(h == HK - 1))
        u0sb = smallp.tile([128, 1], f32)
        nc.vector.tensor_copy(u0sb[:], u0ps[:, :1])
        # u0T [1, 128] then broadcast to all partitions via ones-matmul
        ptt = pua.tile([128, 128], f32, tag="u")
        nc.tensor.transpose(ptt[:1, :], u0sb[:], ident[:])
        u0Tsb = smallp.tile([1, 128], f32)
        nc.vector.tensor_copy(u0Tsb[:], ptt[:1, :])
        ptb = pua.tile([128, 128], f32, tag="u")
        nc.tensor.matmul(ptb[:], ones[0:1, :], u0Tsb[:], start=True, stop=True)
        u0bc = smallp.tile([128, 128], f32)
        nc.vector.tensor_copy(u0bc[:], ptb[:])

        # ------------- probs = relu(Wsv @ u0 + bsv)  [128, FI] --------------
        probs = smallp.tile([128, FI], f32)
        RB = 4  # router batch: 4 Wsv tiles per DVE op pair
        assert FI % RB == 0
        for fb in range(FI // RB):
            wsvt = wsvpool.tile([128, RB * R], f32)
            nc.sync.dma_start(
                wsvt[:].rearrange("p (b r) -> p b r", b=RB),
                t_wsv[128 * RB * fb:128 * RB * (fb + 1), :]
                .rearrange("(b q) r -> q b r", b=RB))
            scr = scrpool.tile([128, RB * R], f32)
            # fused tensor_tensor_reduce crashes this runtime: keep separate
            u0v = u0bc[:, :R].rearrange("p (o r) -> p o r", o=1)
            nc.vector.tensor_tensor(
                scr[:].rearrange("p (b r) -> p b r", b=RB),
                wsvt[:].rearrange("p (b r) -> p b r", b=RB),
                u0v.to_broadcast([128, RB, R]), op=Alu.mult)
            nc.vector.tensor_reduce(
                probs[:, RB * fb:RB * (fb + 1)],
                scr[:].rearrange("p (b r) -> p b r", b=RB),
                axis=Axis.X, op=Alu.add)
        nc.vector.tensor_add(probs[:], probs[:], bsvc[:])
        nc.vector.tensor_scalar_max(probs[:], probs[:], 0.0)
        nc.sync.dma_start(o_probs, probs[:])
        bits = probs[:].bitcast(u32)

        # ------------- bisection for the K-th largest (bit domain) ----------
        lo = smallp.tile([128, 1], u32)
        hi = smallp.tile([128, 1], u32)
        mid = smallp.tile([128, 1], u32)
        ge = smallp.tile([128, FI], f32)
        cnt = smallp.tile([128, 1], f32)
        pred = smallp.tile([128, 1], u32)
        npred = smallp.tile([128, 1], u32)
        nc.vector.memset(lo[:], 0)
        nc.vector.memset(hi[:], 2147483648)
        for _ in range(BISECT_ITERS):
            nc.vector.tensor_tensor(mid[:], lo[:], hi[:], op=Alu.add)
            nc.vector.tensor_scalar(mid[:], mid[:], 1, None,
                                    op0=Alu.logical_shift_right)
            nc.vector.tensor_tensor(ge[:], bits, mid[:].to_broadcast([128, FI]),
                                    op=Alu.is_ge)
            nc.vector.tensor_reduce(cnt[:], ge[:], axis=Axis.X, op=Alu.add)
            totp = pdo.tile([128, 32], f32, tag="d")
            nc.tensor.matmul(totp[:, :1], ones[:], cnt[:], start=True, stop=True)
            nc.vector.tensor_scalar(pred[:], totp[:, :1], float(K), None, op0=Alu.is_ge)
            nc.vector.tensor_scalar(npred[:], totp[:, :1], float(K), None, op0=Alu.is_lt)
            nc.vector.copy_predicated(lo[:], pred[:], mid[:])
            nc.vector.copy_predicated(hi[:], npred[:], mid[:])

        # ------------- local mask + candidate compaction ----------------
        lmask = smallp.tile([128, FL], u32)
        nc.vector.tensor_tensor(lmask[:], probs[:, :FL].bitcast(u32),
                                lo[:].to_broadcast([128, FL]), op=Alu.is_ge)
        cand = smallp.tile([128, FL], f32)
        nc.vector.memset(cand[:], -1.0)
        nc.vector.copy_predicated(cand[:], lmask[:], candidx[:])
        wr16 = smallp.tile([16, 8 * FL], f32)
        for a in range(8):
            nc.sync.dma_start(wr16[0:16, FL * a:FL * (a + 1)],
                              cand[16 * a:16 * (a + 1), :])
        comp = smallp.tile([16, NSLOT // 16], f32)
        nf = smallp.tile([1, 1], u32)
        nc.gpsimd.sparse_gather(comp[:], wr16[:], num_found=nf[:])
        nc.sync.dma_start(o_nf, nf[:])

        # broadcast num_found to all partitions (f32)
        nf_f32 = smallp.tile([1, 1], f32)
        nc.vector.tensor_copy(nf_f32[:], nf[:])
        pnf = pua.tile([128, 128], f32, tag="u")
        nc.tensor.matmul(pnf[:, :1], ones[0:1, :], nf_f32[:], start=True, stop=True)
        nfbc = smallp.tile([128, 1], f32)
        nc.vector.tensor_copy(nfbc[:], pnf[:, :1])

        # pad slots: comp (for DMA) -> 0.0 (real row, finite data);
        # comp2 (for scatter compare) -> 1e9 (matches nothing)
        padm = smallp.tile([16, NSLOT // 16], u32)
        nc.vector.tensor_tensor(padm[:], wiota[:],
                                nfbc[0:16, :].to_broadcast([16, NSLOT // 16]),
                                op=Alu.is_ge)
        comp2 = smallp.tile([16, NSLOT // 16], f32)
        nc.vector.tensor_copy(comp2[:], comp[:])
        zeros16 = smallp.tile([16, NSLOT // 16], f32)
        nc.vector.memset(zeros16[:], 0.0)
        nc.vector.copy_predicated(comp[:], padm[:], zeros16[:])
        big16 = smallp.tile([16, NSLOT // 16], f32)
        nc.vector.memset(big16[:], 1.0e9)
        nc.vector.copy_predicated(comp2[:], padm[:], big16[:])

        comp_u32 = smallp.tile([16, NSLOT // 16], u32)
        nc.vector.tensor_copy(comp_u32[:], comp[:])

        # relayout wrapped slot s=16f+p -> (tile t=f//8, partition 16*(f%8)+p)
        gidx = smallp.tile([128, NT], u32)
        csrc = comp_u32[:].rearrange("p (a b) -> p a b", b=8)
        for bb in range(8):
            nc.sync.dma_start(gidx[16 * bb:16 * (bb + 1), :], csrc[:, :, bb])
        gidxf = smallp.tile([128, NT], f32)
        csrc2 = comp2[:].rearrange("p (a b) -> p a b", b=8)
        for bb in range(8):
            nc.sync.dma_start(gidxf[16 * bb:16 * (bb + 1), :], csrc2[:, :, bb])

        # ------------- gate/up: gather rows, transpose, x-stationary mm -----
        muts = []
        for t in range(NT):
            wgr = wgpool.tile([128, H], f32)
            nc.gpsimd.indirect_dma_start(
                out=wgr[:], out_offset=None, in_=t_wg,
                in_offset=bass.IndirectOffsetOnAxis(ap=gidx[:, t:t + 1], axis=0))
            wur = wupool.tile([128, H], f32)
            nc.gpsimd.indirect_dma_start(
                out=wur[:], out_offset=None, in_=t_wu,
                in_offset=bass.IndirectOffsetOnAxis(ap=gidx[:, t:t + 1], axis=0))
            gps = pga.tile([B, 128], f32, tag="g")
            ups = pua.tile([B, 128], f32, tag="u")
            for h in range(HK):
                p1 = ptr.tile([128, 128], f32, tag="t")
                nc.tensor.transpose(p1[:], wgr[:, 128 * h:128 * h + 128], ident[:])
                wgT = wtpool.tile([128, 128], f32, tag="wt")
                nc.scalar.copy(wgT[:], p1[:])
                p2 = ptr.tile([128, 128], f32, tag="t")
                nc.tensor.transpose(p2[:], wur[:, 128 * h:128 * h + 128], ident[:])
                wuT = wtpool.tile([128, 128], f32, tag="wt")
                nc.vector.tensor_copy(wuT[:], p2[:])
                nc.tensor.matmul(gps[:], xT[:, B * h:B * (h + 1)], wgT[:],
                                 start=(h == 0), stop=(h == HK - 1))
                nc.tensor.matmul(ups[:], xT[:, B * h:B * (h + 1)], wuT[:],
                                 start=(h == 0), stop=(h == HK - 1))
            sig = tmpp.tile([B, 128], f32, tag="sig")
            nc.scalar.activation(sig[:], gps[:], Act.Sigmoid)
            t1 = tmpp.tile([B, 128], f32, tag="t1")
            nc.vector.tensor_tensor(t1[:], gps[:], sig[:], op=Alu.mult)
            mu2 = tmpp.tile([B, 128], f32, tag="mu2")
            nc.vector.tensor_tensor(mu2[:], t1[:], ups[:], op=Alu.mult)
            # transpose mu to [slots, B] for the scatter matmul
            pmt = ptr.tile([128, 128], f32, tag="t")
            nc.tensor.transpose(pmt[:, :B], mu2[:], ident[:B, :B])
            mut = mutpool.tile([128, B], f32, tag="mut")
            nc.vector.tensor_copy(mut[:], pmt[:, :B])
            muts.append(mut)

        # ------------- scatter compact mu to dense local rows ----------------
        muds = []
        for fl in range(FL):
            rowmat = tmpp.tile([128, 128], f32, tag="rowmat")
            nc.vector.tensor_scalar(rowmat[:], col_iota[:], float(128 * fl), None,
                                    op0=Alu.add)
            mdp = ptr.tile([128, 128], f32, tag="t")
            for t in range(NT):
                pblk = tmpp.tile([128, 128], f32, tag="pblk")
                nc.vector.tensor_tensor(pblk[:],
                                        gidxf[:, t:t + 1].to_broadcast([128, 128]),
                                        rowmat[:], op=Alu.is_equal)
                nc.tensor.matmul(mdp[:, :B], pblk[:], muts[t][:],
                                 start=(t == 0), stop=(t == NT - 1))
            mud = mudpool.tile([128, B], f32, tag="mud")
            nc.vector.tensor_copy(mud[:], mdp[:, :B])
            muds.append(mud)

        # ------------- down: stream WdT shard, no transposes ----------------
        oacc = oaccpool.tile([B, H], f32)
        FLC = 2  # psum-accumulate depth over fl
        assert FL % FLC == 0
        for fg in range(FL // FLC):
            wts = []
            for j in range(FLC):
                fl = FLC * fg + j
                wdt = wdtpool.tile([128, H], f32, tag="wdt")
                nc.gpsimd.dma_start(wdt[:], t_wdT[128 * fl:128 * fl + 128, :])
                wts.append((fl, wdt))
            for hb in range(HB):
                dps = pdo.tile([B, 512], f32, tag="d")
                for j, (fl, wdt) in enumerate(wts):
                    nc.tensor.matmul(dps[:], muds[fl][:],
                                     wdt[:, 512 * hb:512 * (hb + 1)],
                                     start=(j == 0), stop=(j == FLC - 1))
                sl = oacc[:, 512 * hb:512 * (hb + 1)]
                if fg == 0:
                    nc.vector.tensor_copy(sl, dps[:])
                else:
                    nc.vector.tensor_add(sl, sl, dps[:])
        nc.sync.dma_start(o_out, oacc[:])

    nc.compile()
    return nc


def make_consts(cfg):
    IS, FI, FL, HK, NT, NSLOT = _derived(cfg)
    p = np.arange(128, dtype=np.float32)[:, None]
    return {
        "ident": np.eye(128, dtype=np.float32),
        "ones": np.ones((128, 128), dtype=np.float32),
        "candidx": (128.0 * np.arange(FL, dtype=np.float32)[None, :] + p)
        .astype(np.float32),
        "wiota": (16.0 * np.arange(NSLOT // 16, dtype=np.float32)[None, :]
                  + np.arange(16, dtype=np.float32)[:, None]).astype(np.float32),
        "col_iota": np.tile(np.arange(128, dtype=np.float32)[None, :], (128, 1)),
    }


def make_in_maps(cfg, x, Wg, Wu, Wd, Wr, Wsv, bsv):
    IS, FI, FL, HK, NT, NSLOT = _derived(cfg)
    consts = make_consts(cfg)
    xT = np.ascontiguousarray(x.T)
    wrT = np.ascontiguousarray(Wr.T)
    in_maps = []
    for c in range(NCORES):
        lo, hi = c * IS, (c + 1) * IS
        wsv_perm = np.concatenate([Wsv[lo:hi], Wsv[:lo], Wsv[hi:]], axis=0)
        bsv_perm = np.concatenate([bsv[lo:hi], bsv[:lo], bsv[hi:]], axis=0)
        bsv_pf = np.ascontiguousarray(bsv_perm.reshape(FI, 128).T)
        m = {
            "xT": xT,
            "wrT": wrT,
            "wsv": np.ascontiguousarray(wsv_perm),
            "bsv_pf": bsv_pf,
            "wg": np.ascontiguousarray(Wg[lo:hi]),
            "wu": np.ascontiguousarray(Wu[lo:hi]),
            "wdT": np.ascontiguousarray(Wd[:, lo:hi].T),
        }
        m.update(consts)
        in_maps.append(m)
    return in_maps


_NC_CACHE = {}


def get_nc(cfg=None):
    cfg = cfg or FULL_CFG
    key = tuple(sorted(cfg.items()))
    if key not in _NC_CACHE:
        _NC_CACHE[key] = _build_nc(cfg)
    return _NC_CACHE[key]


def run_sharded(inputs, trace=False, trace_cores=None, cfg=None):
    """Run the SPMD kernel on 8 cores; returns (out [B, H], BassKernelResults)."""
    from concourse.bass_utils import run_bass_kernel_spmd

    cfg = cfg or FULL_CFG
    nc = get_nc(cfg)
    in_maps = make_in_maps(cfg, inputs["x"], inputs["Wg"], inputs["Wu"],
                           inputs["Wd"], inputs["Wr"], inputs["Wsv"],
                           inputs["bsv"])
    res = run_bass_kernel_spmd(nc, in_maps, list(range(NCORES)),
                               trace=trace, trace_cores=trace_cores)
    acc = None
    for c in range(NCORES):
        part = np.asarray(res.results[c]["out"], dtype=np.float32)
        acc = part if acc is None else acc + part
    return np.ascontiguousarray(acc), res


def kernel(**inputs):
    inputs = {k: np.asarray(v) for k, v in inputs.items()}
    out, _ = run_sharded(inputs, trace=False)
    return out.astype(np.float32)


# revision 11
# speedup vs baseline: 1.3647x; 1.3647x over previous
"""Trainium2 Bass kernel for nn_MistralSparseMLP (topk_masking).

Self-contained: kernel(**inputs) -> np.ndarray takes the FULL inputs
(x, Wg, Wu, Wd, Wr, Wsv, bsv) and returns the FULL [B, H] output.

v3 strategy (8-way tensor parallel over the intermediate dim I, no
collectives):
  - host prep (pure layout, no compute): x.T, Wr.T, per-core Wd-shard
    transposed to [IS, H]; Wsv/bsv permuted per core so the local shard
    is always probs columns [0, FL) (keeps the SPMD program identical);
    index/identity constants.
  - router: u0 = Wr @ x0 on PE (WrT direct from DRAM), probs =
    relu(Wsv.u0 + bsv) via batched DVE mult+reduce (Wsv replicated).
  - global top-K threshold: branchless 31-step bisection on the fp32
    bit patterns, replicated across lanes; cross-partition counts via
    ones-matmul on PE.
  - compaction: candidates -> gpsimd sparse_gather -> NSLOT slots; pads
    get index 0 for the DMA gather (finite data) and 1e9 for the
    scatter compare (never matches => contribute zero).
  - gate/up: indirect-DMA gather of only the selected rows of Wg/Wu
    (the memory win); PE transpose of the gathered blocks; x-stationary
    matmuls accumulate g/u as [B, slots].
  - mu scatter: selection matrices (is_equal of gather indices vs row
    ids) matmul the compact mu back to dense local rows [c, B].
  - down: WdT shard streams dense [c-major] straight from DRAM (no
    on-device transposes), fp32 matmuls accumulate out[B, 512] in psum
    2-deep over fl, DVE/gpsimd adds into a [B, H] SBUF accumulator.
  - host sums the 8 partial [B, H] outputs.
"""
import sys

sys.path.insert(0, "/opt/trn_rl_repo")

import numpy as np

NCORES = 8

FULL_CFG = dict(H=4096, I=14336, R=128, B=32, K=4096, NT=5, WD_BUFS=4)
BISECT_ITERS = 31


def _derived(cfg):
    H, I, B = cfg["H"], cfg["I"], cfg["B"]
    IS = I // NCORES          # local shard of I
    FI = I // 128             # probs free cols
    FL = IS // 128            # local probs cols
    HK = H // 128             # H chunks
    NT = cfg["NT"]
    NSLOT = NT * 128          # padded compact slots
    assert I % (NCORES * 128) == 0 and H % 128 == 0
    return IS, FI, FL, HK, NT, NSLOT


def _build_nc(cfg):
    from contextlib import ExitStack

    import concourse.bass as bass
    import concourse.bacc as bacc
    import concourse.mybir as mybir
    import concourse.tile as tile

    H, I, R, B, K = cfg["H"], cfg["I"], cfg["R"], cfg["B"], cfg["K"]
    IS, FI, FL, HK, NT, NSLOT = _derived(cfg)
    WD_BUFS = cfg["WD_BUFS"]
    HB = H // 512             # down output column blocks

    dt = mybir.dt
    f32, u32 = dt.float32, dt.uint32
    Alu = mybir.AluOpType
    Act = mybir.ActivationFunctionType
    Axis = mybir.AxisListType

    nc = bacc.Bacc("TRN2", target_bir_lowering=False, debug=False)

    def din(name, shape, d=f32):
        return nc.dram_tensor(name, shape, d, kind="ExternalInput").ap()

    def dout(name, shape, d=f32):
        return nc.dram_tensor(name, shape, d, kind="ExternalOutput").ap()

    t_xT = din("xT", [H, B])
    t_wrT = din("wrT", [H, R])
    t_wsv = din("wsv", [I, R])          # per-core permuted: local shard first
    t_bsv = din("bsv_pf", [128, FI])    # permuted + [p, f] relayout
    t_wg = din("wg", [IS, H])
    t_wu = din("wu", [IS, H])
    t_wdT = din("wdT", [IS, H])         # host-transposed Wd shard
    t_ident = din("ident", [128, 128])
    t_ones = din("ones", [128, 128])
    t_candidx = din("candidx", [128, FL])      # f32 value 128*fl + p
    t_wiota = din("wiota", [16, NSLOT // 16])  # f32 wrapped slot id 16*f + p
    t_coliota = din("col_iota", [128, 128])    # f32 value = column index

    o_out = dout("out", [B, H])
    o_nf = dout("nf", [1, 1], u32)
    o_probs = dout("probs", [128, FI])

    with ExitStack() as ctx:
        tc = ctx.enter_context(tile.TileContext(nc))

        def pool(name, bufs, space="SBUF"):
            return ctx.enter_context(tc.tile_pool(name=name, bufs=bufs, space=space))

        cpool = pool("consts", 1)
        xtpool = pool("xt", 1)
        wrpool = pool("wrc", 1)
        wsvpool = pool("wsv", 3)
        scrpool = pool("scr", 2)
        smallp = pool("small", 1)        # persistent small tiles
        tmpp = pool("tmp", 4)            # transient small tiles
        wtpool = pool("wt", 6)           # transposed gate/up blocks
        wgpool = pool("wgraw", 2)
        wupool = pool("wuraw", 2)
        mutpool = pool("mut", NT)
        mudpool = pool("mud", FL)
        wdtpool = pool("wdt", WD_BUFS)
        oaccpool = pool("oacc", 1)

        # PSUM: 8 banks, statically partitioned by pool/tag.
        ptr = pool("ptr", 4, space="PSUM")      # transposes/scatter (tag "t")
        pga = pool("pga", 1, space="PSUM")      # gate accum + u0 (tag "g")
        pua = pool("pua", 1, space="PSUM")      # up accum + bcasts (tag "u")
        pdo = pool("pdo", 2, space="PSUM")      # down accum + bisect (tag "d")

        # ---------------- constants ----------------
        ident = cpool.tile([128, 128], f32)
        nc.sync.dma_start(ident[:], t_ident)
        ones = cpool.tile([128, 128], f32)
        nc.sync.dma_start(ones[:], t_ones)
        candidx = cpool.tile([128, FL], f32)
        nc.sync.dma_start(candidx[:], t_candidx)
        wiota = cpool.tile([16, NSLOT // 16], f32)
        nc.sync.dma_start(wiota[:], t_wiota)
        bsvc = cpool.tile([128, FI], f32)
        nc.sync.dma_start(bsvc[:], t_bsv)
        col_iota = cpool.tile([128, 128], f32)
        nc.sync.dma_start(col_iota[:], t_coliota)

        # ---------------- xT  [128, HK*B] direct from host x.T -------------
        xT = xtpool.tile([128, HK * B], f32)
        nc.sync.dma_start(xT[:].rearrange("p (h b) -> p h b", h=HK),
                          t_xT.rearrange("(h p) b -> p h b", p=128))

        # ---------------- u0 = Wr @ x0  [R=128, 1] ----------------
        wrall = wrpool.tile([128, HK * R], f32)
        nc.sync.dma_start(wrall[:].rearrange("p (h r) -> p h r", h=HK),
                          t_wrT.rearrange("(h p) r -> p h r", p=128))
        u0ps = pga.tile([128, 32], f32, tag="g")
        for h in range(HK):
            nc.tensor.matmul(u0ps[:, :1], wrall[:, R * h:R * (h + 1)],
                             xT[:, B * h:B * h + 1],
                             start=(h == 0), stop=(h == HK - 1))
        u0sb = smallp.tile([128, 1], f32)
        nc.vector.tensor_copy(u0sb[:], u0ps[:, :1])
        # u0T [1, 128] then broadcast to all partitions via ones-matmul
        ptt = pua.tile([128, 128], f32, tag="u")
        nc.tensor.transpose(ptt[:1, :], u0sb[:], ident[:])
        u0Tsb = smallp.tile([1, 128], f32)
        nc.vector.tensor_copy(u0Tsb[:], ptt[:1, :])
        ptb = pua.tile([128, 128], f32, tag="u")
        nc.tensor.matmul(ptb[:], ones[0:1, :], u0Tsb[:], start=True, stop=True)
        u0bc = smallp.tile([128, 128], f32)
        nc.vector.tensor_copy(u0bc[:], ptb[:])

        # ------------- probs = relu(Wsv @ u0 + bsv)  [128, FI] --------------
        probs = smallp.tile([128, FI], f32)
        RB = 8  # router batch: 8 Wsv tiles per DVE op pair
        assert FI % RB == 0
        for fb in range(FI // RB):
            wsvt = wsvpool.tile([128, RB * R], f32)
            nc.sync.dma_start(
                wsvt[:].rearrange("p (b r) -> p b r", b=RB),
                t_wsv[128 * RB * fb:128 * RB * (fb + 1), :]
                .rearrange("(b q) r -> q b r", b=RB))
            scr = scrpool.tile([128, RB * R], f32)
            # fused tensor_tensor_reduce crashes this runtime: keep separate
            u0v = u0bc[:, :R].rearrange("p (o r) -> p o r", o=1)
            nc.vector.tensor_tensor(
                scr[:].rearrange("p (b r) -> p b r", b=RB),
                wsvt[:].rearrange("p (b r) -> p b r", b=RB),
                u0v.to_broadcast([128, RB, R]), op=Alu.mult)
            nc.vector.tensor_reduce(
                probs[:, RB * fb:RB * (fb + 1)],
                scr[:].rearrange("p (b r) -> p b r", b=RB),
                axis=Axis.X, op=Alu.add)
        nc.vector.tensor_add(probs[:], probs[:], bsvc[:])
        nc.vector.tensor_scalar_max(probs[:], probs[:], 0.0)
        nc.sync.dma_start(o_probs, probs[:])
        bits = probs[:].bitcast(u32)

        # ------------- bisection for the K-th largest (bit domain) ----------
        lo = smallp.tile([128, 1], u32)
        hi = smallp.tile([128, 1], u32)
        mid = smallp.tile([128, 1], u32)
        ge = smallp.tile([128, FI], f32)
        cnt = smallp.tile([128, 1], f32)
        pred = smallp.tile([128, 1], u32)
        npred = smallp.tile([128, 1], u32)
        nc.vector.memset(lo[:], 0)
        nc.vector.memset(hi[:], 2147483648)
        for _ in range(BISECT_ITERS):
            nc.vector.tensor_tensor(mid[:], lo[:], hi[:], op=Alu.add)
            nc.vector.tensor_scalar(mid[:], mid[:], 1, None,
                                    op0=Alu.logical_shift_right)
            nc.vector.tensor_tensor(ge[:], bits, mid[:].to_broadcast([128, FI]),
                                    op=Alu.is_ge)
            nc.vector.tensor_reduce(cnt[:], ge[:], axis=Axis.X, op=Alu.add)
            totp = pdo.tile([128, 32], f32, tag="d")
            nc.tensor.matmul(totp[:, :1], ones[:], cnt[:], start=True, stop=True)
            nc.vector.tensor_scalar(pred[:], totp[:, :1], float(K), None, op0=Alu.is_ge)
            nc.vector.tensor_scalar(npred[:], totp[:, :1], float(K), None, op0=Alu.is_lt)
            nc.vector.copy_predicated(lo[:], pred[:], mid[:])
            nc.vector.copy_predicated(hi[:], npred[:], mid[:])

        # ------------- local mask + candidate compaction ----------------
        lmask = smallp.tile([128, FL], u32)
        nc.vector.tensor_tensor(lmask[:], probs[:, :FL].bitcast(u32),
                                lo[:].to_broadcast([128, FL]), op=Alu.is_ge)
        cand = smallp.tile([128, FL], f32)
        nc.vector.memset(cand[:], -1.0)
        nc.vector.copy_predicated(cand[:], lmask[:], candidx[:])
        wr16 = smallp.tile([16, 8 * FL], f32)
        for a in range(8):
            nc.sync.dma_start(wr16[0:16, FL * a:FL * (a + 1)],
                              cand[16 * a:16 * (a + 1), :])
        comp = smallp.tile([16, NSLOT // 16], f32)
        nf = smallp.tile([1, 1], u32)
        nc.gpsimd.sparse_gather(comp[:], wr16[:], num_found=nf[:])
        nc.sync.dma_start(o_nf, nf[:])

        # broadcast num_found to all partitions (f32)
        nf_f32 = smallp.tile([1, 1], f32)
        nc.vector.tensor_copy(nf_f32[:], nf[:])
        pnf = pua.tile([128, 128], f32, tag="u")
        nc.tensor.matmul(pnf[:, :1], ones[0:1, :], nf_f32[:], start=True, stop=True)
        nfbc = smallp.tile([128, 1], f32)
        nc.vector.tensor_copy(nfbc[:], pnf[:, :1])

        # pad slots: comp (for DMA) -> 0.0 (real row, finite data);
        # comp2 (for scatter compare) -> 1e9 (matches nothing)
        padm = smallp.tile([16, NSLOT // 16], u32)
        nc.vector.tensor_tensor(padm[:], wiota[:],
                                nfbc[0:16, :].to_broadcast([16, NSLOT // 16]),
                                op=Alu.is_ge)
        comp2 = smallp.tile([16, NSLOT // 16], f32)
        nc.vector.tensor_copy(comp2[:], comp[:])
        zeros16 = smallp.tile([16, NSLOT // 16], f32)
        nc.vector.memset(zeros16[:], 0.0)
        nc.vector.copy_predicated(comp[:], padm[:], zeros16[:])
        big16 = smallp.tile([16, NSLOT // 16], f32)
        nc.vector.memset(big16[:], 1.0e9)
        nc.vector.copy_predicated(comp2[:], padm[:], big16[:])

        comp_u32 = smallp.tile([16, NSLOT // 16], u32)
        nc.vector.tensor_copy(comp_u32[:], comp[:])

        # relayout wrapped slot s=16f+p -> (tile t=f//8, partition 16*(f%8)+p)
        gidx = smallp.tile([128, NT], u32)
        csrc = comp_u32[:].rearrange("p (a b) -> p a b", b=8)
        for bb in range(8):
            nc.sync.dma_start(gidx[16 * bb:16 * (bb + 1), :], csrc[:, :, bb])
        gidxf = smallp.tile([128, NT], f32)
        csrc2 = comp2[:].rearrange("p (a b) -> p a b", b=8)
        for bb in range(8):
            nc.sync.dma_start(gidxf[16 * bb:16 * (bb + 1), :], csrc2[:, :, bb])

        # ------------- gate/up: gather rows, transpose, x-stationary mm -----
        muts = []
        for t in range(NT):
            wgr = wgpool.tile([128, H], f32)
            nc.gpsimd.indirect_dma_start(
                out=wgr[:], out_offset=None, in_=t_wg,
                in_offset=bass.IndirectOffsetOnAxis(ap=gidx[:, t:t + 1], axis=0))
            wur = wupool.tile([128, H], f32)
            nc.gpsimd.indirect_dma_start(
                out=wur[:], out_offset=None, in_=t_wu,
                in_offset=bass.IndirectOffsetOnAxis(ap=gidx[:, t:t + 1], axis=0))
            gps = pga.tile([B, 128], f32, tag="g")
            ups = pua.tile([B, 128], f32, tag="u")
            for h in range(HK):
                p1 = ptr.tile([128, 128], f32, tag="t")
                nc.tensor.transpose(p1[:], wgr[:, 128 * h:128 * h + 128], ident[:])
                wgT = wtpool.tile([128, 128], f32, tag="wt")
                nc.scalar.copy(wgT[:], p1[:])
                p2 = ptr.tile([128, 128], f32, tag="t")
                nc.tensor.transpose(p2[:], wur[:, 128 * h:128 * h + 128], ident[:])
                wuT = wtpool.tile([128, 128], f32, tag="wt")
                nc.vector.tensor_copy(wuT[:], p2[:])
                nc.tensor.matmul(gps[:], xT[:, B * h:B * (h + 1)], wgT[:],
                                 start=(h == 0), stop=(h == HK - 1))
                nc.tensor.matmul(ups[:], xT[:, B * h:B * (h + 1)], wuT[:],
                                 start=(h == 0), stop=(h == HK - 1))
            sig = tmpp.tile([B, 128], f32, tag="sig")
            nc.scalar.activation(sig[:], gps[:], Act.Sigmoid)
            t1 = tmpp.tile([B, 128], f32, tag="t1")
            nc.vector.tensor_tensor(t1[:], gps[:], sig[:], op=Alu.mult)
            mu2 = tmpp.tile([B, 128], f32, tag="mu2")
            nc.vector.tensor_tensor(mu2[:], t1[:], ups[:], op=Alu.mult)
            # transpose mu to [slots, B] for the scatter matmul
            pmt = ptr.tile([128, 128], f32, tag="t")
            nc.tensor.transpose(pmt[:, :B], mu2[:], ident[:B, :B])
            mut = mutpool.tile([128, B], f32, tag="mut")
            nc.vector.tensor_copy(mut[:], pmt[:, :B])
            muts.append(mut)

        # ------------- scatter compact mu to dense local rows ----------------
        muds = []
        for fl in range(FL):
            rowmat = tmpp.tile([128, 128], f32, tag="rowmat")
            nc.vector.tensor_scalar(rowmat[:], col_iota[:], float(128 * fl), None,
                                    op0=Alu.add)
            mdp = ptr.tile([128, 128], f32, tag="t")
            for t in range(NT):
                pblk = tmpp.tile([128, 128], f32, tag="pblk")
                nc.vector.tensor_tensor(pblk[:],
                                        gidxf[:, t:t + 1].to_broadcast([128, 128]),
                                        rowmat[:], op=Alu.is_equal)
                nc.tensor.matmul(mdp[:, :B], pblk[:], muts[t][:],
                                 start=(t == 0), stop=(t == NT - 1))
            mud = mudpool.tile([128, B], f32, tag="mud")
            nc.vector.tensor_copy(mud[:], mdp[:, :B])
            muds.append(mud)

        # ------------- down: stream WdT shard, no transposes ----------------
        oacc = oaccpool.tile([B, H], f32)
        FLC = 2  # psum-accumulate depth over fl
        assert FL % FLC == 0
        for fg in range(FL // FLC):
            wts = []
            for j in range(FLC):
                fl = FLC * fg + j
                wdt = wdtpool.tile([128, H], f32, tag="wdt")
                nc.gpsimd.dma_start(wdt[:], t_wdT[128 * fl:128 * fl + 128, :])
                wts.append((fl, wdt))
            for hb in range(HB):
                dps = pdo.tile([B, 512], f32, tag="d")
                for j, (fl, wdt) in enumerate(wts):
                    nc.tensor.matmul(dps[:], muds[fl][:],
                                     wdt[:, 512 * hb:512 * (hb + 1)],
                                     start=(j == 0), stop=(j == FLC - 1))
                sl = oacc[:, 512 * hb:512 * (hb + 1)]
                if fg == 0:
                    nc.vector.tensor_copy(sl, dps[:])
                else:
                    nc.vector.tensor_add(sl, sl, dps[:])
        nc.sync.dma_start(o_out, oacc[:])

    nc.compile()
    return nc


def make_consts(cfg):
    IS, FI, FL, HK, NT, NSLOT = _derived(cfg)
    p = np.arange(128, dtype=np.float32)[:, None]
    return {
        "ident": np.eye(128, dtype=np.float32),
        "ones": np.ones((128, 128), dtype=np.float32),
        "candidx": (128.0 * np.arange(FL, dtype=np.float32)[None, :] + p)
        .astype(np.float32),
        "wiota": (16.0 * np.arange(NSLOT // 16, dtype=np.float32)[None, :]
                  + np.arange(16, dtype=np.float32)[:, None]).astype(np.float32),
        "col_iota": np.tile(np.arange(128, dtype=np.float32)[None, :], (128, 1)),
    }


def make_in_maps(cfg, x, Wg, Wu, Wd, Wr, Wsv, bsv):
    IS, FI, FL, HK, NT, NSLOT = _derived(cfg)
    consts = make_consts(cfg)
    xT = np.ascontiguousarray(x.T)
    wrT = np.ascontiguousarray(Wr.T)
    in_maps = []
    for c in range(NCORES):
        lo, hi = c * IS, (c + 1) * IS
        wsv_perm = np.concatenate([Wsv[lo:hi], Wsv[:lo], Wsv[hi:]], axis=0)
        bsv_perm = np.concatenate([bsv[lo:hi], bsv[:lo], bsv[hi:]], axis=0)
        bsv_pf = np.ascontiguousarray(bsv_perm.reshape(FI, 128).T)
        m = {
            "xT": xT,
            "wrT": wrT,
            "wsv": np.ascontiguousarray(wsv_perm),
            "bsv_pf": bsv_pf,
            "wg": np.ascontiguousarray(Wg[lo:hi]),
            "wu": np.ascontiguousarray(Wu[lo:hi]),
            "wdT": np.ascontiguousarray(Wd[:, lo:hi].T),
        }
        m.update(consts)
        in_maps.append(m)
    return in_maps


_NC_CACHE = {}


def get_nc(cfg=None):
    cfg = cfg or FULL_CFG
    key = tuple(sorted(cfg.items()))
    if key not in _NC_CACHE:
        _NC_CACHE[key] = _build_nc(cfg)
    return _NC_CACHE[key]


def run_sharded(inputs, trace=False, trace_cores=None, cfg=None):
    """Run the SPMD kernel on 8 cores; returns (out [B, H], BassKernelResults)."""
    from concourse.bass_utils import run_bass_kernel_spmd

    cfg = cfg or FULL_CFG
    nc = get_nc(cfg)
    in_maps = make_in_maps(cfg, inputs["x"], inputs["Wg"], inputs["Wu"],
                           inputs["Wd"], inputs["Wr"], inputs["Wsv"],
                           inputs["bsv"])
    res = run_bass_kernel_spmd(nc, in_maps, list(range(NCORES)),
                               trace=trace, trace_cores=trace_cores)
    acc = None
    for c in range(NCORES):
        part = np.asarray(res.results[c]["out"], dtype=np.float32)
        acc = part if acc is None else acc + part
    return np.ascontiguousarray(acc), res


def kernel(**inputs):
    inputs = {k: np.asarray(v) for k, v in inputs.items()}
    out, _ = run_sharded(inputs, trace=False)
    return out.astype(np.float32)
